# revision 6
# baseline (speedup 1.0000x reference)
"""Multi-head attention (no mask, no 1/sqrt(dk) scaling) on 8 Trainium2 cores.

Reference computation (B=2, N=2048, D=1024, H=16, DK=DV=64):
    Q = X @ Wq + bq ; K = X @ Wk + bk ; V = X @ Wv + bv   (per-head split)
    A = softmax(Q K^T)  (full, unmasked, unscaled)
    out = (A V concat heads) @ WO

Sharding: tensor-parallel over heads — 2 heads per core, both batches on
every core.  Each core computes its heads' Q/K/V projections, attention,
and the partial output projection through its 128 rows of WO; the host sums
the 8 partial outputs (the all-reduce of row-parallel WO).

Layout strategy (per core):
  -  X is fed pre-transposed (XT [D, B*N]) so every matmul contracts along
     SBUF partitions with no on-device transposes of X.
  -  Q^T, K^T are built [head_dim(128), token(4096)]; scores are computed
     transposed (S^T [keys, queries]) so softmax denominators come out of
     the AV matmul itself via a ones-column appended to V.
  -  exp() runs on the scalar engine directly out of PSUM.
  -  All matmuls run in float32r (full PE rate at ~1e-4 relative error).
  -  The output projection is computed transposed (OUT^T [D, tokens]) with
     WO chunks stationary; the host transposes back.
"""

import os
import numpy as np

B, N, D, H, DK, DV = 2, 2048, 1024, 16, 64, 64
NCORES = 8
HPC = H // NCORES            # heads per core
HD = HPC * DK                # 128: per-core head dims (2 heads x 64)
T = B * N                    # 4096 tokens
NB = 512                     # projection token-block
IB = 512                     # attention query-block
P = 128

LAST_RESULT = None           # BassKernelResults of the most recent run (for test.py)


def _build_module():
    import concourse.bacc as bacc
    import concourse.mybir as mybir
    import concourse.tile as tile
    from concourse.masks import make_identity

    f32 = mybir.dt.float32
    f32r = mybir.dt.float32r
    Exp = mybir.ActivationFunctionType.Exp
    mult = mybir.AluOpType.mult

    def r(ap):
        return ap.bitcast(f32r)

    nc = bacc.Bacc("TRN2", target_bir_lowering=False)

    dXT = nc.dram_tensor("XT", [D, T], f32, kind="ExternalInput")
    dWQ = nc.dram_tensor("WQ", [D, HD], f32, kind="ExternalInput")
    dWK = nc.dram_tensor("WK", [D, HD], f32, kind="ExternalInput")
    dWV = nc.dram_tensor("WV", [D, HD], f32, kind="ExternalInput")
    dBQ = nc.dram_tensor("BQ", [HD, 1], f32, kind="ExternalInput")
    dBK = nc.dram_tensor("BK", [HD, 1], f32, kind="ExternalInput")
    dBV = nc.dram_tensor("BV", [HD, 1], f32, kind="ExternalInput")
    dWO = nc.dram_tensor("WO", [HD, D], f32, kind="ExternalInput")
    dOUTT = nc.dram_tensor("OUTT", [D, T], f32, kind="ExternalOutput")

    KD = D // P              # 8 contraction chunks for the projections
    NPB = T // NB            # 8 projection blocks
    NIB = N // IB            # 4 query blocks per batch
    NJC = N // P             # 16 key chunks per batch
    NTC = T // P             # 32 token chunks

    xt_v = dXT[:, :].rearrange("(kc p) t -> p kc t", p=P)     # [128, 8, T]
    wq_v = dWQ[:, :].rearrange("(kc p) m -> p kc m", p=P)
    wk_v = dWK[:, :].rearrange("(kc p) m -> p kc m", p=P)
    wv_v = dWV[:, :].rearrange("(kc p) m -> p kc m", p=P)

    with tile.TileContext(nc) as tc:
        with tc.tile_pool(name="singles", bufs=1) as singles:
            wq = singles.tile([P, KD, HD], f32r, tag="wq")
            wk = singles.tile([P, KD, HD], f32r, tag="wk")
            wv = singles.tile([P, KD, HD], f32r, tag="wv")
            nc.sync.dma_start(wq[:], wq_v.bitcast(f32r))
            nc.sync.dma_start(wk[:], wk_v.bitcast(f32r))
            nc.sync.dma_start(wv[:], wv_v.bitcast(f32r))
            bq = singles.tile([HD, 1], f32, tag="bq")
            bk = singles.tile([HD, 1], f32, tag="bk")
            bv = singles.tile([HD, 1], f32, tag="bv")
            nc.sync.dma_start(bq[:], dBQ[:, :])
            nc.sync.dma_start(bk[:], dBK[:, :])
            nc.sync.dma_start(bv[:], dBV[:, :])
            wo = singles.tile([HD, D], f32r, tag="wo")
            nc.sync.dma_start(wo[:], dWO[:, :].bitcast(f32r))
            ident = singles.tile([P, P], f32, tag="ident")
            make_identity(nc, ident[:])

            qt = singles.tile([P, T], f32r, tag="qt")       # Q^T [hd, tok]
            kt = singles.tile([P, T], f32r, tag="kt")       # K^T [hd, tok]
            # V in natural layout + ones column, per head: [tok%128, chunk, 65]
            vg0 = singles.tile([P, NTC, DV + 1], f32r, tag="vg0")
            vg1 = singles.tile([P, NTC, DV + 1], f32r, tag="vg1")
            nc.gpsimd.memset(vg0[:].bitcast(f32), 1.0)
            nc.gpsimd.memset(vg1[:].bitcast(f32), 1.0)
            # normalized AV^T, both heads stacked: [hd(128), tok]
            avn = singles.tile([P, T], f32r, tag="avn")

            # ---------------- phase 1: projections + V transpose ----------
            with tc.tile_pool(name="p12sb", bufs=1) as p12sb, \
                 tc.tile_pool(name="p12ps", bufs=2, space="PSUM") as p12ps:
                vt = p12sb.tile([P, T], f32, tag="vt")     # V^T staging
                for nb in range(NPB):
                    ts = slice(nb * NB, (nb + 1) * NB)
                    xb = p12sb.tile([P, KD, NB], f32r, tag="xb", bufs=2)
                    nc.sync.dma_start(xb[:], xt_v[:, :, ts].bitcast(f32r))
                    for w_sb, b_sb, dst in ((wq, bq, qt), (wk, bk, kt),
                                            (wv, bv, vt)):
                        pp = p12ps.tile([P, NB], f32, tag="proj")
                        for kc in range(KD):
                            nc.tensor.matmul(pp[:], w_sb[:, kc, :],
                                             xb[:, kc, :],
                                             start=(kc == 0),
                                             stop=(kc == KD - 1))
                        # evacuate + bias (per-partition scalar add)
                        nc.vector.tensor_scalar_add(dst[:, ts], pp[:], b_sb[:])

                # V^T -> V natural layout, split per head, via PE transpose
                for tc_i in range(NTC):
                    cs = slice(tc_i * P, (tc_i + 1) * P)
                    pt_ps = p12ps.tile([P, P], f32, tag="tr")
                    nc.tensor.transpose(pt_ps[:], vt[:, cs], ident[:])
                    nc.vector.tensor_copy(out=vg0[:, tc_i, 0:DV],
                                          in_=pt_ps[:, 0:DV])
                    nc.vector.tensor_copy(out=vg1[:, tc_i, 0:DV],
                                          in_=pt_ps[:, DV:2 * DV])

            # ---------------- phase 2: attention -------------------------
            with tc.tile_pool(name="attnsb", bufs=1) as attnsb, \
                 tc.tile_pool(name="drsc", bufs=4, space="DRAM") as drsc, \
                 tc.tile_pool(name="attnps", bufs=1, space="PSUM") as attnps:
                for b in range(B):
                    for ib in range(NIB):
                        i_sl = slice(b * N + ib * IB, b * N + (ib + 1) * IB)
                        pts = []
                        for h in range(HPC):
                            hs = slice(h * DK, (h + 1) * DK)
                            pt = attnsb.tile([P, NJC, IB], f32r,
                                             tag=f"pt{h}", bufs=1)
                            pts.append(pt)
                            for jc in range(NJC):
                                j_sl = slice(b * N + jc * P,
                                             b * N + (jc + 1) * P)
                                ps_s = attnps.tile([P, IB], f32, tag="s",
                                                   bufs=3)
                                # S^T[j, i] = K^T[d, j].T @ Q^T[d, i]
                                nc.tensor.matmul(ps_s[:], kt[hs, j_sl],
                                                 qt[hs, i_sl],
                                                 start=True, stop=True)
                                nc.scalar.activation(out=pt[:, jc, :],
                                                     in_=ps_s[:], func=Exp)
                        for h in range(HPC):
                            vg = vg0 if h == 0 else vg1
                            pt = pts[h]
                            # AV^T (+ denominator row) accumulated over keys
                            ps_av = attnps.tile([DV + 1, IB], f32,
                                                tag=f"av{h}", bufs=1)
                            for jc in range(NJC):
                                nc.tensor.matmul(
                                    ps_av[:], vg[:, b * (N // P) + jc, :],
                                    pt[:, jc, :],
                                    start=(jc == 0), stop=(jc == NJC - 1))
                            rt = attnsb.tile([DV + 1, IB], f32, tag="rt",
                                             bufs=2)
                            nc.vector.reciprocal(rt[DV:DV + 1, :],
                                                 ps_av[DV:DV + 1, :])
                            dr = drsc.tile([1, IB], f32, tag="dr", bufs=4)
                            nc.sync.dma_start(dr[:], rt[DV:DV + 1, :])
                            bc = attnsb.tile([DV, IB], f32, tag="bc", bufs=2)
                            nc.gpsimd.dma_start(
                                bc[:], dr[0:1, :].to_broadcast([DV, IB]))
                            if h == 0:
                                nc.vector.tensor_tensor(
                                    out=avn[0:DV, i_sl], in0=ps_av[0:DV, :],
                                    in1=bc[:], op=mult)
                            else:
                                tmp = attnsb.tile([DV, IB], f32r, tag="tmp1",
                                                  bufs=2)
                                nc.vector.tensor_tensor(
                                    out=tmp[:], in0=ps_av[0:DV, :],
                                    in1=bc[:], op=mult)
                                nc.sync.dma_start(avn[DV:2 * DV, i_sl],
                                                  tmp[:])

                # ------------ phase 3: output projection (transposed) -----
                for dc in range(D // P):
                    ds_ = slice(dc * P, (dc + 1) * P)
                    for tb in range(T // NB):
                        tbs = slice(tb * NB, (tb + 1) * NB)
                        pw = attnps.tile([P, NB], f32, tag="wo", bufs=3)
                        nc.tensor.matmul(pw[:], wo[:, ds_],
                                         avn[:, tbs],
                                         start=True, stop=True)
                        ob = attnsb.tile([P, NB], f32, tag="ob", bufs=3)
                        nc.vector.tensor_copy(out=ob[:], in_=pw[:])
                        nc.sync.dma_start(dOUTT[ds_, tbs], ob[:])

    nc.finalize()
    return nc


_MODULE = None


def kernel(X, Wq, bq, Wk, bk, Wv, bv, WO):
    global _MODULE, LAST_RESULT
    from concourse.bass_utils import run_bass_kernel_spmd

    X = np.asarray(X, dtype=np.float32)
    XT = np.ascontiguousarray(X.reshape(T, D).T)            # [D, T]
    Wq = np.asarray(Wq, dtype=np.float32)
    Wk = np.asarray(Wk, dtype=np.float32)
    Wv = np.asarray(Wv, dtype=np.float32)
    WO = np.asarray(WO, dtype=np.float32)
    bq = np.asarray(bq, dtype=np.float32)
    bk = np.asarray(bk, dtype=np.float32)
    bv = np.asarray(bv, dtype=np.float32)

    if _MODULE is None:
        _MODULE = _build_module()

    in_maps = []
    for c in range(NCORES):
        hsl = slice(c * HD, (c + 1) * HD)
        in_maps.append({
            "XT": XT,
            "WQ": np.ascontiguousarray(Wq[:, hsl]),
            "WK": np.ascontiguousarray(Wk[:, hsl]),
            "WV": np.ascontiguousarray(Wv[:, hsl]),
            "BQ": np.ascontiguousarray(bq[hsl]).reshape(HD, 1),
            "BK": np.ascontiguousarray(bk[hsl]).reshape(HD, 1),
            "BV": np.ascontiguousarray(bv[hsl]).reshape(HD, 1),
            "WO": np.ascontiguousarray(WO[hsl, :]),
        })

    trace = bool(int(os.environ.get("KERNEL_TRACE", "0")))
    LAST_RESULT = run_bass_kernel_spmd(
        _MODULE, in_maps, core_ids=list(range(NCORES)), trace=trace)

    out_t = np.zeros((D, T), dtype=np.float64)
    for res in LAST_RESULT.results:
        out_t += res["OUTT"].astype(np.float64)
    return np.ascontiguousarray(out_t.T).astype(np.float32).reshape(B, N, D)
